# revision 11
# baseline (speedup 1.0000x reference)
"""GAT-style message passing kernel for Trainium2, 8-core row-parallel SPMD.

Math (reference):
  g = (h @ W).reshape(N, H, F)                     # N=1024, H=4, F=32
  e[i,j,h]   = sum_f a_w[f] * lrelu(g[i,h,f] + g[j,h,f])
  att        = softmax_j(e)
  hiddens    = mean_h(att @ g)                     # [N, F]
  amean      = mean_h(att); amean = clip(amean / (rowmax(amean)[j] + 1e-10), 0, 1)

Key transform: lrelu(x) = 0.6*x + 0.4*|x| (slope 0.2), so
  e[i,j,h] = 0.6*al[i,h] + 0.6*al[j,h] + 0.4 * sum_f a_f |g_i + g_j|
with al[n,h] = sum_f a_f g[n,h,f]. The 0.6*al[i,h] term is constant along j
and cancels in the softmax, so it is dropped. The |g_i + g_j| tensor is
computed per destination row i as one [128(hf), 1024(j)] op on ACT (Abs with
per-partition bias) or DVE (tensor_scalar add + abs_max 0), and the weighted
f-sum is a PE matmul with a block-diagonal stationary built from a_w.

Sharding: core c owns destination rows i in [128c, 128c+128). Inputs h/W/a_w
are replicated; per-core hcT selects the row block. Outputs per core: raw
amean rows, per-row max M, and hiddens rows. The final column normalization
amean[i,j] / (M[j]+1e-10) needs all-core M, done on host (1024 floats).
"""

import sys

sys.path.insert(0, "/opt/trn_rl_repo")

import numpy as np  # noqa: E402

import concourse.bass as bass  # noqa: E402
import concourse.bacc as bacc  # noqa: E402
import concourse.tile as tile  # noqa: E402
import concourse.mybir as mybir  # noqa: E402
from concourse import bass_utils  # noqa: E402

F32 = mybir.dt.float32
N = 1024
CORES = 8
R = N // CORES  # 128 destination rows per core
H = 4
F = 32
HF = H * F  # 128
NB = 4  # batches of 32 rows per core
BI = R // NB  # 32 i-rows per batch

# fraction of abs ops on ACT vs DVE (ACT is 1.2GHz, DVE 0.96GHz; ACT also does exp)
ACT_SHARE = 5  # out of 9: il % 9 < 5 -> ACT


def _act_func(name):
    return getattr(mybir.ActivationFunctionType, name)


def build_nc():
    nc = bacc.Bacc("TRN2", target_bir_lowering=False, debug=False, num_devices=1)

    d_hT = nc.dram_tensor("hT", [128, N], F32, kind="ExternalInput")
    d_hcT = nc.dram_tensor("hcT", [128, R], F32, kind="ExternalInput")
    d_W = nc.dram_tensor("W", [128, HF], F32, kind="ExternalInput")
    d_aw = nc.dram_tensor("aw", [1, F], F32, kind="ExternalInput")

    d_am = nc.dram_tensor("am_raw", [R, N], F32, kind="ExternalOutput")
    d_M = nc.dram_tensor("M", [R, 1], F32, kind="ExternalOutput")
    d_hid = nc.dram_tensor("hid", [R, F], F32, kind="ExternalOutput")

    # constants
    hsel_np = np.zeros((H, 128), np.float32)
    for m in range(128):
        hsel_np[m % H, m] = 1.0
    sum4_np = np.zeros((128, BI), np.float32)
    for p in range(128):
        sum4_np[p, p // H] = 0.25
    ident_np = np.eye(128, dtype=np.float32)
    d_hsel = nc.inline_tensor(hsel_np, "hsel")
    d_sum4 = nc.inline_tensor(sum4_np, "sum4")
    d_ident = nc.inline_tensor(ident_np, "ident")

    AF = mybir.ActivationFunctionType
    AX = mybir.AxisListType
    ALU = mybir.AluOpType

    with tile.TileContext(nc) as tc:
        with (
            tc.tile_pool(name="singles", bufs=1) as singles,
            tc.tile_pool(name="absp", bufs=4) as absp,
            tc.tile_pool(name="attp", bufs=2) as attp,
            tc.tile_pool(name="tp", bufs=3) as tp,
            tc.tile_pool(name="statp", bufs=4) as statp,
            tc.tile_pool(name="ps_e", bufs=4, space="PSUM") as ps_e,
            tc.tile_pool(name="ps_t", bufs=2, space="PSUM") as ps_t,
            tc.tile_pool(name="ps_hid", bufs=2, space="PSUM") as ps_hid,
        ):
            # ---- load inputs ----
            sb_hT = singles.tile([128, N], F32)
            sb_hcT = singles.tile([128, R], F32)
            sb_W = singles.tile([128, HF], F32)
            sb_hsel = singles.tile([H, 128], F32)
            sb_sum4 = singles.tile([128, BI], F32)
            sb_ident = singles.tile([128, 128], F32)
            nc.sync.dma_start(sb_hT[:], d_hT.ap())
            nc.sync.dma_start(sb_hcT[:], d_hcT.ap())
            nc.sync.dma_start(sb_W[:], d_W.ap())
            nc.sync.dma_start(sb_hsel[:], d_hsel.ap())
            nc.sync.dma_start(sb_sum4[:], d_sum4.ap())
            nc.sync.dma_start(sb_ident[:], d_ident.ap())

            # Ablk[(32h+f), m] = a_w[f] if m == h else 0   [128, 4]
            sb_Ablk = singles.tile([128, H], F32)
            nc.vector.memset(sb_Ablk[:], 0.0)
            for h in range(H):
                nc.sync.dma_start(
                    sb_Ablk[F * h : F * h + F, h : h + 1],
                    d_aw.ap().rearrange("a b -> b a"),
                )
            sb_Ablk08n = singles.tile([128, H], F32)
            nc.scalar.mul(sb_Ablk08n[:], sb_Ablk[:], 0.8)

            # Apad: 32 stationary blocks [128, 128]; block il has 0.8*Ablk at
            # columns 4*il..4*il+3 (so matmul writes psum rows 4*il..4*il+3)
            sb_Apad = singles.tile([128, BI * 128], F32)
            nc.vector.memset(sb_Apad[:], 0.0)
            for il in range(BI):
                c0 = 128 * il + 4 * il
                nc.vector.tensor_copy(sb_Apad[:, c0 : c0 + 4], sb_Ablk08n[:])

            # ---- gT = (h @ W).T : [hf, n], and its negation ----
            sb_gT = singles.tile([128, N], F32)
            sb_gTneg = singles.tile([128, N], F32)
            for s in range(2):
                ps = ps_e.tile([128, 512], F32, tag="e")
                nc.tensor.matmul(
                    ps[:], sb_W[:], sb_hT[:, 512 * s : 512 * s + 512],
                    start=True, stop=True,
                )
                nc.vector.tensor_copy(sb_gT[:, 512 * s : 512 * s + 512], ps[:])
                nc.scalar.mul(sb_gTneg[:, 512 * s : 512 * s + 512], ps[:], -1.0)

            # gTcneg = -(hc @ W).T : [hf, i_local] (neg bias columns per local i)
            sb_gTcneg = singles.tile([128, R], F32)
            ps = ps_t.tile([128, 128], F32, tag="t")
            nc.tensor.matmul(ps[:], sb_W[:], sb_hcT[:], start=True, stop=True)
            nc.scalar.mul(sb_gTcneg[:], ps[:], -1.0)

            # g rows scaled by 0.25 (head-mean folded): chunk t = rows 128t..
            sb_g025 = singles.tile([128, 8, 128], F32)
            for t in range(8):
                ps = ps_t.tile([128, 128], F32, tag="t")
                nc.tensor.transpose(ps[:], sb_gT[:, 128 * t : 128 * t + 128], sb_ident[:])
                nc.scalar.mul(sb_g025[:, t, :], ps[:], 0.25)

            # alT[h, n] = sum_f a_f gT[(h,f), n]   [4, 1024]
            sb_alT = singles.tile([H, N], F32)
            for s in range(2):
                ps = ps_t.tile([H, 512], F32, tag="t")
                nc.tensor.matmul(
                    ps[:], sb_Ablk[:], sb_gT[:, 512 * s : 512 * s + 512],
                    start=True, stop=True,
                )
                nc.vector.tensor_copy(sb_alT[:, 512 * s : 512 * s + 512], ps[:])

            # outputs accumulated in SBUF
            sb_amean = singles.tile([R, N], F32)
            sb_hid = singles.tile([R, F], F32)

            # ---- main loop: 4 batches of 32 destination rows ----
            for b in range(NB):
                e0 = ps_e.tile([128, 512], F32, tag="e")
                e1 = ps_e.tile([128, 512], F32, tag="e")
                eh = (e0, e1)
                # preload linear term al[j,h] into all 128 rows
                for s in range(2):
                    nc.tensor.matmul(
                        eh[s][:], sb_hsel[:], sb_alT[:, 512 * s : 512 * s + 512],
                        start=True, stop=False, skip_group_check=True,
                    )
                for il in range(BI):
                    ir = BI * b + il
                    sb_abs = absp.tile([128, N], F32, tag="abs")
                    nbias_col = sb_gTcneg[:, ir : ir + 1]
                    if il % 9 < ACT_SHARE:
                        nc.scalar.activation(
                            sb_abs[:], sb_gT[:], AF.Relu, bias=nbias_col, scale=-1.0
                        )
                    else:
                        nc.vector.tensor_scalar(
                            sb_abs[:], sb_gTneg[:], nbias_col, 0.0,
                            op0=ALU.add, op1=ALU.max,
                        )
                    for s in range(2):
                        nc.tensor.matmul(
                            eh[s][:],
                            sb_Apad[:, 128 * il : 128 * il + 128],
                            sb_abs[:, 512 * s : 512 * s + 512],
                            start=False, stop=(il == BI - 1),
                            skip_group_check=True,
                        )

                # softmax over j (free dim), rows are (i_local, h)
                sb_mx = statp.tile([128, 1], F32, tag="mx")
                sb_negmx = statp.tile([128, 1], F32, tag="negmx")
                sb_D = statp.tile([128, 1], F32, tag="D")
                sb_rD = statp.tile([128, 1], F32, tag="rD")
                sb_att = attp.tile([128, N], F32, tag="att")
                sb_attn = attp.tile([128, N], F32, tag="attn")
                nc.vector.reduce_max(sb_mx[:, 0:1], e0[:], axis=AX.X)
                sb_mx2 = statp.tile([128, 1], F32, tag="mx2")
                nc.vector.reduce_max(sb_mx2[:, 0:1], e1[:], axis=AX.X)
                nc.vector.tensor_tensor(
                    sb_mx[:, 0:1], sb_mx[:, 0:1], sb_mx2[:, 0:1], op=ALU.max
                )
                nc.scalar.mul(sb_negmx[:], sb_mx[:], -1.0)
                # exp(e - mx), accumulate row sums
                sb_D2 = statp.tile([128, 1], F32, tag="D2")
                nc.scalar.activation(
                    sb_att[:, 0:512], e0[:], AF.Exp,
                    bias=sb_negmx[:, 0:1], scale=1.0, accum_out=sb_D[:, 0:1],
                )
                nc.scalar.activation(
                    sb_att[:, 512:1024], e1[:], AF.Exp,
                    bias=sb_negmx[:, 0:1], scale=1.0, accum_out=sb_D2[:, 0:1],
                )
                nc.vector.tensor_tensor(
                    sb_D[:, 0:1], sb_D[:, 0:1], sb_D2[:, 0:1], op=ALU.add
                )
                nc.vector.reciprocal(sb_rD[:, 0:1], sb_D[:, 0:1])
                nc.vector.tensor_scalar(
                    sb_attn[:], sb_att[:], sb_rD[:, 0:1], None, op0=ALU.mult
                )

                # amean rows: 0.25 * sum_h attn  -> [32, 1024]
                for s in range(2):
                    psa = ps_t.tile([BI, 512], F32, tag="t")
                    nc.tensor.matmul(
                        psa[:], sb_sum4[:], sb_attn[:, 512 * s : 512 * s + 512],
                        start=True, stop=True,
                    )
                    nc.vector.tensor_copy(
                        sb_amean[BI * b : BI * b + BI, 512 * s : 512 * s + 512],
                        psa[:],
                    )

                # hiddens rows: transpose attn chunks, per-head matmul vs g025
                psum_hid = ps_hid.tile([BI, F], F32, tag="hid")
                for t in range(8):
                    pst = ps_t.tile([128, 128], F32, tag="t")
                    nc.tensor.transpose(
                        pst[:], sb_attn[:, 128 * t : 128 * t + 128], sb_ident[:]
                    )
                    sb_aT = tp.tile([128, 128], F32, tag="aT")
                    nc.vector.tensor_copy(sb_aT[:], pst[:])
                    for h in range(H):
                        nc.tensor.matmul(
                            psum_hid[:],
                            sb_aT[:, h :: H],
                            sb_g025[:, t, F * h : F * h + F],
                            start=(t == 0 and h == 0),
                            stop=(t == 7 and h == H - 1),
                            skip_group_check=True,
                        )
                nc.vector.tensor_copy(
                    sb_hid[BI * b : BI * b + BI, :], psum_hid[:]
                )

            # per-row max of raw amean
            sb_M = singles.tile([R, 1], F32)
            nc.vector.reduce_max(sb_M[:, 0:1], sb_amean[:], axis=AX.X)

            nc.sync.dma_start(d_am.ap(), sb_amean[:])
            nc.sync.dma_start(d_M.ap(), sb_M[:])
            nc.sync.dma_start(d_hid.ap(), sb_hid[:])

    nc.compile()
    return nc


_NC_CACHE = {}


def _get_nc():
    if "nc" not in _NC_CACHE:
        _NC_CACHE["nc"] = build_nc()
    return _NC_CACHE["nc"]


def kernel(h, W, a_w, _trace=False):
    h = np.asarray(h, np.float32)
    W = np.asarray(W, np.float32)
    a_w = np.asarray(a_w, np.float32)
    nc = _get_nc()

    hT = np.ascontiguousarray(h.T)
    aw2 = np.ascontiguousarray(a_w.reshape(1, F))
    in_maps = []
    for c in range(CORES):
        in_maps.append(
            {
                "hT": hT,
                "hcT": np.ascontiguousarray(h[R * c : R * c + R].T),
                "W": W,
                "aw": aw2,
            }
        )
    res = bass_utils.run_bass_kernel_spmd(
        nc, in_maps, core_ids=list(range(CORES)), trace=_trace
    )
    kernel.last_results = res

    amean_raw = np.concatenate([res.results[c]["am_raw"] for c in range(CORES)], axis=0)
    Mv = np.concatenate([res.results[c]["M"][:, 0] for c in range(CORES)], axis=0)
    hiddens = np.concatenate([res.results[c]["hid"] for c in range(CORES)], axis=0)

    r = 1.0 / (Mv + 1e-10)
    a_mean = np.clip(amean_raw * r[None, :], 0.0, 1.0).astype(np.float32)
    return hiddens, a_mean
